# revision 16
# baseline (speedup 1.0000x reference)
"""Trainium2 Bass kernel for nn_ExpandingLinear.

Computation (see reference):
    x_exp = concat([x, x[:, p0] * v0, x_exp1[:, p1] * v1], axis=1)   # [B, 2176]
    W     = scatter_add(weight_vals at [weight_rows, weight_cols])    # [2048, 2176]
    b     = scatter_add(bias_vals at bias_idx)                        # [2048]
    out   = x_exp @ W.T + b                                           # [B, 2048]

Key transform: every expanded feature is x[:, q_j] * a_j for a base column
q_j (embed chains resolved on host), so its weight column can be folded into
the base weight matrix: W'[:, q_j] += a_j * W[:, 2048+j]. The device then
computes a plain dense out = x @ W'^T + b with K = 2048 (16 k-tiles of 128).

Sharding: data-parallel over the batch dim across 8 NeuronCores (1024 rows
per core); W'/bias replicated. Host prep is sharding + parameter/layout work
(batch split, transpose, COO densification, fold, bf16 conversion); the
full dense matmul and bias add run on device.

Device kernel (per core), bf16 operands, fp32 PSUM accumulation:
  - xk[k]   [128, 1024]  x^T k-slice (feature-part, batch-free)
  - wt      [128, 2048]  W'^T chunks (4 k-tiles x 512 out-cols)
  - out[m*128:.., n*512:..] = sum_k xk[k][:,m].T @ w(k,n) + bias  (8 m-tiles,
    4 n-blocks of 512, 8 single-bank PSUM accumulators)
  - round 0 (n=0) runs k-outer/m-inner so matmul #1 needs only a 32KB x
    sliver + 128KB w sliver (both split out as dedicated first DMAs); each
    k-step's data arrives well ahead of the PE.  Rounds 1-3 run
    m-outer/k-outer/n-inner with all weights SBUF-resident, so psum
    evacuations stagger through the rounds.
  - outputs collect per m-tile in SBUF and store as single 512KB DMAs; the
    last m-tile stores per-n right after each evacuation so the final
    store's completion latency overlaps the remaining work.
  - outputs are stored bf16 (host upcasts); max rel err ~3e-3 vs fp32 ref.
"""

import numpy as np
from contextlib import ExitStack

OUT = 2048
IN_BASE = 2048
N_EMBED = 64
IN_TOT = IN_BASE + 2 * N_EMBED  # 2176
BATCH = 8192
N_CORES = 8
B_CORE = BATCH // N_CORES       # 1024
P = 128
K_TILES = IN_BASE // P          # 16 (embed columns folded into base W)
M_TILES = B_CORE // P           # 8
N_SPLIT = 4                     # 2048 out cols in 4 x 512 (one PSUM bank each)
NW = 512
K2 = K_TILES // 2               # host wt layout granularity: 2 k-tiles
K4 = K_TILES // 4               # device wt chunk granularity: 4 k-tiles

_CACHED = {}


def _build_nc():
    import concourse.bass as bass
    import concourse.mybir as mybir
    import concourse.tile as tile
    from concourse import bacc

    f32 = mybir.dt.float32
    f16 = mybir.dt.bfloat16

    nc = bacc.Bacc("TRN2", target_bir_lowering=False, debug=False,
                   num_devices=N_CORES)

    xk = nc.dram_tensor("xk", [K_TILES, P, B_CORE], f16, kind="ExternalInput")
    wt = nc.dram_tensor("wt", [K2, N_SPLIT, P, 2 * NW], f16,
                        kind="ExternalInput")
    bias = nc.dram_tensor("bias", [P, OUT], f32, kind="ExternalInput")
    out = nc.dram_tensor("out", [M_TILES, P, OUT], f16,
                         kind="ExternalOutput")

    with tile.TileContext(nc) as tc:
        with ExitStack() as ctx:
            xk_pool = ctx.enter_context(tc.tile_pool(name="xk", bufs=K_TILES))
            wt_pool = ctx.enter_context(tc.tile_pool(name="wt", bufs=32))
            small_pool = ctx.enter_context(tc.tile_pool(name="small", bufs=1))
            out_pool = ctx.enter_context(tc.tile_pool(name="out", bufs=8))
            psum_pool = ctx.enter_context(
                tc.tile_pool(name="psum", bufs=8, space="PSUM"))

            # ---- DMA streams (per-queue FIFO order == consumption order)
            xk_tiles = [None] * K_TILES
            wck_tiles = [[None] * N_SPLIT for _ in range(K2)]

            def load_xk(k, eng):
                t = xk_pool.tile([P, B_CORE], f16, tag="xk", name=f"xk{k}")
                eng.dma_start(out=t[:], in_=xk.ap()[k])
                xk_tiles[k] = t

            def load_wck(k2i, n, eng):
                t = wt_pool.tile([P, 2 * NW], f16, tag="wck",
                                 name=f"wck_k{k2i}_n{n}")
                eng.dma_start(out=t[:], in_=wt.ap()[k2i, n])
                wck_tiles[k2i][n] = t

            load_xk(0, nc.sync)
            load_wck(0, 0, nc.scalar)
            # k1 gates matmul #9 at ~13.7us; a full 256KB tile on one queue
            # lands ~15us (1.4us stall, and the idle re-throttles the PE
            # clock).  Split it across both HWDGE queues (SWDGE starts too
            # late to help): each half lands ~12us.
            x1a = small_pool.tile([P, B_CORE // 2], f16, tag="x1a")
            nc.sync.dma_start(out=x1a[:], in_=xk.ap()[1][:, 0:B_CORE // 2])
            x1b = small_pool.tile([P, B_CORE // 2], f16, tag="x1b")
            nc.scalar.dma_start(out=x1b[:], in_=xk.ap()[1][:, B_CORE // 2:])
            for k in range(2, K_TILES, 2):
                load_xk(k, nc.sync)
            for k in range(3, K_TILES, 2):
                load_xk(k, nc.scalar)
            for k2i in range(1, K2):
                load_wck(k2i, 0, nc.gpsimd)
            for k2i in range(K2):
                load_wck(k2i, 1, nc.sync)
            for k2i in range(K2):
                load_wck(k2i, 2, nc.scalar)
            for k2i in range(K2 // 2):
                load_wck(k2i, 3, nc.sync)
            for k2i in range(K2 // 2, K2):
                load_wck(k2i, 3, nc.scalar)

            bias_t = small_pool.tile([P, OUT], f32, tag="bias")
            nc.gpsimd.dma_start(out=bias_t[:], in_=bias.ap())

            def lhs(k, m):
                if k == 1:
                    h, mi = divmod(m, M_TILES // 2)
                    t = x1b if h else x1a
                    return t[:, mi * P:(mi + 1) * P]
                return xk_tiles[k][:, m * P:(m + 1) * P]

            def mm(psum, k, m, n):
                nc.tensor.matmul(
                    psum[:],
                    lhsT=lhs(k, m),
                    rhs=wck_tiles[k // 2][n][:, (k % 2) * NW:(k % 2 + 1) * NW],
                    start=(k == 0),
                    stop=(k == K_TILES - 1),
                )

            # round 0 (n=0), k-outer/m-inner: DMA-paced startup.
            psums0 = [psum_pool.tile([P, NW], f32, tag="ps",
                                     name=f"ps_n0_m{m}")
                      for m in range(M_TILES)]
            for k in range(K_TILES):
                for m in range(M_TILES):
                    mm(psums0[m], k, m, 0)

            ot_m = [out_pool.tile([P, OUT], f16, tag="otm", name=f"ot{m}")
                    for m in range(M_TILES - 1)]
            for m in range(M_TILES - 1):
                nc.vector.tensor_add(ot_m[m][:, 0:NW], psums0[m][:],
                                     bias_t[:, 0:NW])
            ot7_0 = out_pool.tile([P, NW], f16, tag="ot7", name="ot7_0")
            nc.vector.tensor_add(ot7_0[:], psums0[M_TILES - 1][:],
                                 bias_t[:, 0:NW])
            nc.sync.dma_start(out=out.ap()[M_TILES - 1][:, 0:NW],
                              in_=ot7_0[:])

            # rounds 1-3 merged: m-outer/k-outer/n-inner, weights resident.
            for m in range(M_TILES):
                psums = [psum_pool.tile([P, NW], f32, tag="ps",
                                        name=f"ps_m{m}_n{n}")
                         for n in range(1, N_SPLIT)]
                for k in range(K_TILES):
                    for n in range(1, N_SPLIT):
                        mm(psums[n - 1], k, m, n)
                if m < M_TILES - 1:
                    for n in range(1, N_SPLIT):
                        nc.vector.tensor_add(
                            ot_m[m][:, n * NW:(n + 1) * NW],
                            psums[n - 1][:], bias_t[:, n * NW:(n + 1) * NW])
                    nc.gpsimd.dma_start(out=out.ap()[m], in_=ot_m[m][:])
                else:
                    # last m-tile: store per n right after each evacuation so
                    # the stores' ~2us completion latency overlaps the rest.
                    st_eng = [nc.scalar, nc.sync, nc.scalar]
                    for n in range(1, N_SPLIT):
                        ot = out_pool.tile([P, NW], f16, tag="ot7",
                                           name=f"ot7_{n}")
                        nc.vector.tensor_add(
                            ot[:], psums[n - 1][:],
                            bias_t[:, n * NW:(n + 1) * NW])
                        st_eng[n - 1].dma_start(
                            out=out.ap()[m][:, n * NW:(n + 1) * NW],
                            in_=ot[:])

    nc.compile()
    return nc


def _host_prep(inputs):
    x = np.asarray(inputs["x"], dtype=np.float32)
    wv = np.asarray(inputs["weight_vals"], dtype=np.float64)
    wr = np.asarray(inputs["weight_rows"]).astype(np.int64)
    wc = np.asarray(inputs["weight_cols"]).astype(np.int64)
    bv = np.asarray(inputs["bias_vals"], dtype=np.float64)
    bi = np.asarray(inputs["bias_idx"]).astype(np.int64)
    e0v = np.asarray(inputs["embed0_vals"], dtype=np.float64)
    e0p = np.asarray(inputs["embed0_parents"]).astype(np.int64)
    e1v = np.asarray(inputs["embed1_vals"], dtype=np.float64)
    e1p = np.asarray(inputs["embed1_parents"]).astype(np.int64)

    # dense W [OUT, IN_TOT] (coalesce: duplicates sum)
    W = np.bincount(wr * IN_TOT + wc, weights=wv,
                    minlength=OUT * IN_TOT).reshape(OUT, IN_TOT)

    # resolve embed parent chains to (base column, multiplier)
    q = np.empty(2 * N_EMBED, dtype=np.int64)
    a = np.empty(2 * N_EMBED, dtype=np.float64)
    q[:N_EMBED] = e0p
    a[:N_EMBED] = e0v
    for j in range(N_EMBED):
        p = int(e1p[j])
        if p < IN_BASE:
            q[N_EMBED + j] = p
            a[N_EMBED + j] = e1v[j]
        else:
            q[N_EMBED + j] = e0p[p - IN_BASE]
            a[N_EMBED + j] = e1v[j] * e0v[p - IN_BASE]

    # fold embed weight columns into the base weight matrix
    Wf = W[:, :IN_BASE].copy()
    np.add.at(Wf.T, q, (a[None, :] * W[:, IN_BASE:]).T)

    import ml_dtypes
    # wt[k2, n, p, kk*512+j] = Wf.T[(2*k2+kk)*128+p, n*512+j]
    wt = np.ascontiguousarray(
        Wf.T.reshape(K2, 2, P, N_SPLIT, NW)
          .transpose(0, 3, 2, 1, 4).reshape(K2, N_SPLIT, P, 2 * NW)
          .astype(np.float32).astype(ml_dtypes.bfloat16))

    b = np.bincount(bi, weights=bv, minlength=OUT).astype(np.float32)
    bias_bcast = np.ascontiguousarray(
        np.broadcast_to(b[None, :], (P, OUT)).astype(np.float32))

    xks = []
    for i in range(N_CORES):
        xs = x[i * B_CORE:(i + 1) * B_CORE]
        # xk[k, p, :] = xs.T[k*128+p, :]
        xks.append(np.ascontiguousarray(
            xs.T.astype(ml_dtypes.bfloat16).reshape(K_TILES, P, B_CORE)))
    return xks, wt, bias_bcast


def kernel(**inputs) -> np.ndarray:
    import time
    from concourse.bass_utils import run_bass_kernel_spmd

    if "nc" not in _CACHED:
        _CACHED["nc"] = _build_nc()
    nc = _CACHED["nc"]

    xks, wt, bias_bcast = _host_prep(inputs)
    in_maps = [dict(xk=xks[i], wt=wt, bias=bias_bcast)
               for i in range(N_CORES)]
    res = None
    last_exc = None
    for attempt in range(3):
        try:
            res = run_bass_kernel_spmd(nc, in_maps,
                                       core_ids=list(range(N_CORES)))
            break
        except Exception as e:  # transient device/runtime hiccups
            last_exc = e
            time.sleep(2.0)
    if res is None:
        raise last_exc
    parts = [np.asarray(res.results[i]["out"]).reshape(B_CORE, OUT)
             for i in range(N_CORES)]
    return np.concatenate(parts, axis=0).astype(np.float32)


# revision 18
# speedup vs baseline: 1.0424x; 1.0424x over previous
"""Trainium2 Bass kernel for nn_ExpandingLinear.

Computation (see reference):
    x_exp = concat([x, x[:, p0] * v0, x_exp1[:, p1] * v1], axis=1)   # [B, 2176]
    W     = scatter_add(weight_vals at [weight_rows, weight_cols])    # [2048, 2176]
    b     = scatter_add(bias_vals at bias_idx)                        # [2048]
    out   = x_exp @ W.T + b                                           # [B, 2048]

Key transform: every expanded feature is x[:, q_j] * a_j for a base column
q_j (embed chains resolved on host), so its weight column can be folded into
the base weight matrix: W'[:, q_j] += a_j * W[:, 2048+j]. The device then
computes a plain dense out = x @ W'^T + b with K = 2048 (16 k-tiles of 128).

Sharding: data-parallel over the batch dim across 8 NeuronCores (1024 rows
per core); W'/bias replicated. Host prep is sharding + parameter/layout work
(batch split, transpose, COO densification, fold, bf16 conversion); the
full dense matmul and bias add run on device.

Device kernel (per core), bf16 operands, fp32 PSUM accumulation:
  - xk[k]   [128, 1024]  x^T k-slice (feature-part, batch-free)
  - wt      [128, 2048]  W'^T chunks (4 k-tiles x 512 out-cols)
  - out[m*128:.., n*512:..] = sum_k xk[k][:,m].T @ w(k,n) + bias  (8 m-tiles,
    4 n-blocks of 512, 8 single-bank PSUM accumulators)
  - round 0 (n=0) runs k-outer/m-inner so matmul #1 needs only a 32KB x
    sliver + 128KB w sliver (both split out as dedicated first DMAs); each
    k-step's data arrives well ahead of the PE.  Rounds 1-3 run
    m-outer/k-outer/n-inner with all weights SBUF-resident, so psum
    evacuations stagger through the rounds.
  - outputs collect per m-tile in SBUF and store as single 512KB DMAs; the
    last m-tile stores per-n right after each evacuation so the final
    store's completion latency overlaps the remaining work.
  - outputs are stored bf16 (host upcasts); max rel err ~3e-3 vs fp32 ref.
"""

import numpy as np
from contextlib import ExitStack

OUT = 2048
IN_BASE = 2048
N_EMBED = 64
IN_TOT = IN_BASE + 2 * N_EMBED  # 2176
BATCH = 8192
N_CORES = 8
B_CORE = BATCH // N_CORES       # 1024
P = 128
K_TILES = IN_BASE // P          # 16 (embed columns folded into base W)
M_TILES = B_CORE // P           # 8
N_SPLIT = 4                     # 2048 out cols in 4 x 512 (one PSUM bank each)
NW = 512
K2 = K_TILES // 2               # host wt layout granularity: 2 k-tiles
K4 = K_TILES // 4               # device wt chunk granularity: 4 k-tiles

_CACHED = {}


def _build_nc():
    import concourse.bass as bass
    import concourse.mybir as mybir
    import concourse.tile as tile
    from concourse import bacc
    from concourse.tile_rust import add_dep_helper

    f32 = mybir.dt.float32
    f16 = mybir.dt.bfloat16

    nc = bacc.Bacc("TRN2", target_bir_lowering=False, debug=False,
                   num_devices=N_CORES)

    xk = nc.dram_tensor("xk", [K_TILES, P, B_CORE], f16, kind="ExternalInput")
    wt = nc.dram_tensor("wt", [K2, N_SPLIT, P, 2 * NW], f16,
                        kind="ExternalInput")
    bias = nc.dram_tensor("bias", [P, OUT], f32, kind="ExternalInput")
    out = nc.dram_tensor("out", [M_TILES, P, OUT], f16,
                         kind="ExternalOutput")

    with tile.TileContext(nc) as tc:
        with ExitStack() as ctx:
            xk_pool = ctx.enter_context(tc.tile_pool(name="xk", bufs=K_TILES))
            wt_pool = ctx.enter_context(tc.tile_pool(name="wt", bufs=32))
            small_pool = ctx.enter_context(tc.tile_pool(name="small", bufs=1))
            out_pool = ctx.enter_context(tc.tile_pool(name="out", bufs=8))
            psum_pool = ctx.enter_context(
                tc.tile_pool(name="psum", bufs=8, space="PSUM"))

            # ---- DMA streams (per-queue FIFO order == consumption order)
            xk_tiles = [None] * K_TILES
            xk_dmas = [None] * K_TILES
            wck_tiles = [[None] * N_SPLIT for _ in range(K2)]

            def load_xk(k, eng):
                t = xk_pool.tile([P, B_CORE], f16, tag="xk", name=f"xk{k}")
                xk_dmas[k] = eng.dma_start(out=t[:], in_=xk.ap()[k])
                xk_tiles[k] = t

            def load_wck(k2i, n, eng, after=None):
                t = wt_pool.tile([P, 2 * NW], f16, tag="wck",
                                 name=f"wck_k{k2i}_n{n}")
                di = eng.dma_start(out=t[:], in_=wt.ap()[k2i, n])
                if after is not None:
                    # pace the prefetch: the SWDGE queue otherwise hogs the
                    # shared SDMA engines early and starves the HWDGE rings
                    # carrying the round-0-critical x slices.
                    add_dep_helper(di.ins, after.ins, sync=True,
                                   reason="wt prefetch yields to x stream")
                wck_tiles[k2i][n] = t
                return di

            load_xk(0, nc.sync)
            load_wck(0, 0, nc.scalar)
            # k1 gates matmul #9 at ~13.7us; a full 256KB tile on one queue
            # lands too late (and the stall re-throttles the PE clock).
            # Split it across both HWDGE queues.
            x1a = small_pool.tile([P, B_CORE // 2], f16, tag="x1a")
            nc.sync.dma_start(out=x1a[:], in_=xk.ap()[1][:, 0:B_CORE // 2])
            x1b = small_pool.tile([P, B_CORE // 2], f16, tag="x1b")
            nc.scalar.dma_start(out=x1b[:], in_=xk.ap()[1][:, B_CORE // 2:])
            for k in range(2, K_TILES, 2):
                load_xk(k, nc.sync)
            for k in range(3, K_TILES, 2):
                load_xk(k, nc.scalar)
            # n0 chunks feed round 0 at one per ~3.5us; gate each on an x
            # slice that lands ~2 k-steps earlier so the SWDGE drip matches
            # consumption instead of bursting.
            load_wck(1, 0, nc.gpsimd)
            for k2i in range(2, K2):
                load_wck(k2i, 0, nc.gpsimd, after=xk_dmas[2 * k2i - 4])
            for k2i in range(K2):
                load_wck(k2i, 1, nc.sync, after=xk_dmas[14] if k2i == 0
                         else None)
            for k2i in range(K2):
                load_wck(k2i, 2, nc.scalar, after=xk_dmas[15] if k2i == 0
                         else None)
            for k2i in range(K2 // 2):
                load_wck(k2i, 3, nc.sync)
            for k2i in range(K2 // 2, K2):
                load_wck(k2i, 3, nc.scalar)

            bias_t = small_pool.tile([P, OUT], f32, tag="bias")
            bias_dma = nc.gpsimd.dma_start(out=bias_t[:], in_=bias.ap())
            add_dep_helper(bias_dma.ins, xk_dmas[12].ins, sync=True,
                           reason="bias yields to x stream")

            def lhs(k, m):
                if k == 1:
                    h, mi = divmod(m, M_TILES // 2)
                    t = x1b if h else x1a
                    return t[:, mi * P:(mi + 1) * P]
                return xk_tiles[k][:, m * P:(m + 1) * P]

            def mm(psum, k, m, n):
                nc.tensor.matmul(
                    psum[:],
                    lhsT=lhs(k, m),
                    rhs=wck_tiles[k // 2][n][:, (k % 2) * NW:(k % 2 + 1) * NW],
                    start=(k == 0),
                    stop=(k == K_TILES - 1),
                )

            # round 0 (n=0), k-outer/m-inner: DMA-paced startup.
            psums0 = [psum_pool.tile([P, NW], f32, tag="ps",
                                     name=f"ps_n0_m{m}")
                      for m in range(M_TILES)]
            for k in range(K_TILES):
                for m in range(M_TILES):
                    mm(psums0[m], k, m, 0)

            ot_m = [out_pool.tile([P, OUT], f16, tag="otm", name=f"ot{m}")
                    for m in range(M_TILES - 1)]
            for m in range(M_TILES - 1):
                nc.vector.tensor_add(ot_m[m][:, 0:NW], psums0[m][:],
                                     bias_t[:, 0:NW])
            ot7_0 = out_pool.tile([P, NW], f16, tag="ot7", name="ot7_0")
            nc.vector.tensor_add(ot7_0[:], psums0[M_TILES - 1][:],
                                 bias_t[:, 0:NW])
            nc.sync.dma_start(out=out.ap()[M_TILES - 1][:, 0:NW],
                              in_=ot7_0[:])

            # rounds 1-3 merged: m-outer/k-outer/n-inner, weights resident.
            for m in range(M_TILES):
                psums = [psum_pool.tile([P, NW], f32, tag="ps",
                                        name=f"ps_m{m}_n{n}")
                         for n in range(1, N_SPLIT)]
                for k in range(K_TILES):
                    for n in range(1, N_SPLIT):
                        mm(psums[n - 1], k, m, n)
                if m < M_TILES - 1:
                    for n in range(1, N_SPLIT):
                        nc.vector.tensor_add(
                            ot_m[m][:, n * NW:(n + 1) * NW],
                            psums[n - 1][:], bias_t[:, n * NW:(n + 1) * NW])
                    nc.gpsimd.dma_start(out=out.ap()[m], in_=ot_m[m][:])
                else:
                    # last m-tile: store per n right after each evacuation so
                    # the stores' ~2us completion latency overlaps the rest.
                    st_eng = [nc.scalar, nc.sync, nc.scalar]
                    for n in range(1, N_SPLIT):
                        ot = out_pool.tile([P, NW], f16, tag="ot7",
                                           name=f"ot7_{n}")
                        nc.vector.tensor_add(
                            ot[:], psums[n - 1][:],
                            bias_t[:, n * NW:(n + 1) * NW])
                        st_eng[n - 1].dma_start(
                            out=out.ap()[m][:, n * NW:(n + 1) * NW],
                            in_=ot[:])

    nc.compile()
    return nc


def _host_prep(inputs):
    x = np.asarray(inputs["x"], dtype=np.float32)
    wv = np.asarray(inputs["weight_vals"], dtype=np.float64)
    wr = np.asarray(inputs["weight_rows"]).astype(np.int64)
    wc = np.asarray(inputs["weight_cols"]).astype(np.int64)
    bv = np.asarray(inputs["bias_vals"], dtype=np.float64)
    bi = np.asarray(inputs["bias_idx"]).astype(np.int64)
    e0v = np.asarray(inputs["embed0_vals"], dtype=np.float64)
    e0p = np.asarray(inputs["embed0_parents"]).astype(np.int64)
    e1v = np.asarray(inputs["embed1_vals"], dtype=np.float64)
    e1p = np.asarray(inputs["embed1_parents"]).astype(np.int64)

    # dense W [OUT, IN_TOT] (coalesce: duplicates sum)
    W = np.bincount(wr * IN_TOT + wc, weights=wv,
                    minlength=OUT * IN_TOT).reshape(OUT, IN_TOT)

    # resolve embed parent chains to (base column, multiplier)
    q = np.empty(2 * N_EMBED, dtype=np.int64)
    a = np.empty(2 * N_EMBED, dtype=np.float64)
    q[:N_EMBED] = e0p
    a[:N_EMBED] = e0v
    for j in range(N_EMBED):
        p = int(e1p[j])
        if p < IN_BASE:
            q[N_EMBED + j] = p
            a[N_EMBED + j] = e1v[j]
        else:
            q[N_EMBED + j] = e0p[p - IN_BASE]
            a[N_EMBED + j] = e1v[j] * e0v[p - IN_BASE]

    # fold embed weight columns into the base weight matrix
    Wf = W[:, :IN_BASE].copy()
    np.add.at(Wf.T, q, (a[None, :] * W[:, IN_BASE:]).T)

    import ml_dtypes
    # wt[k2, n, p, kk*512+j] = Wf.T[(2*k2+kk)*128+p, n*512+j]
    wt = np.ascontiguousarray(
        Wf.T.reshape(K2, 2, P, N_SPLIT, NW)
          .transpose(0, 3, 2, 1, 4).reshape(K2, N_SPLIT, P, 2 * NW)
          .astype(np.float32).astype(ml_dtypes.bfloat16))

    b = np.bincount(bi, weights=bv, minlength=OUT).astype(np.float32)
    bias_bcast = np.ascontiguousarray(
        np.broadcast_to(b[None, :], (P, OUT)).astype(np.float32))

    xks = []
    for i in range(N_CORES):
        xs = x[i * B_CORE:(i + 1) * B_CORE]
        # xk[k, p, :] = xs.T[k*128+p, :]
        xks.append(np.ascontiguousarray(
            xs.T.astype(ml_dtypes.bfloat16).reshape(K_TILES, P, B_CORE)))
    return xks, wt, bias_bcast


def kernel(**inputs) -> np.ndarray:
    import time
    from concourse.bass_utils import run_bass_kernel_spmd

    if "nc" not in _CACHED:
        _CACHED["nc"] = _build_nc()
    nc = _CACHED["nc"]

    xks, wt, bias_bcast = _host_prep(inputs)
    in_maps = [dict(xk=xks[i], wt=wt, bias=bias_bcast)
               for i in range(N_CORES)]
    res = None
    last_exc = None
    for attempt in range(3):
        try:
            res = run_bass_kernel_spmd(nc, in_maps,
                                       core_ids=list(range(N_CORES)))
            break
        except Exception as e:  # transient device/runtime hiccups
            last_exc = e
            time.sleep(2.0)
    if res is None:
        raise last_exc
    parts = [np.asarray(res.results[i]["out"]).reshape(B_CORE, OUT)
             for i in range(N_CORES)]
    return np.concatenate(parts, axis=0).astype(np.float32)
